# revision 2
# baseline (speedup 1.0000x reference)
"""Trainium2 Bass kernel v2 for nn_MeshLoss2D (chamfer distance between a
point cloud and a bilinearly-refined mesh).

Contract: kernel(vertices, pc) takes FULL inputs, returns the FULL (scalar)
output. Internally shards across 8 NeuronCores (data-parallel over pc rows).

  chamfer = mean_p min_q d(p,q) + mean_q min_p d(p,q),
  d(p,q) = |a_p|^2 + |b_q|^2 - 2 a_p . b_q

Design (per core, per iteration):
  queries = its 1024 pc rows x 2 batches as 16 row-tiles of 128;
  candidates = the full 9056-padded mesh (9025 refined points + 31 pads).
  Distances via K=13 compensated-bf16 matmuls (hi/lo split).  Each row-tile's
  [128, 9056] f32 PSUM is egressed to fp16 SBUF in five groups:
  4x2048 on ACT + the 864 tail on DVE (this split balances the two engines).
  The fp16 copy E is consumed twice:
    AB (pc->mesh rowmin): one DVE tensor_scalar min-accum (4x_2p fast mode).
    BA (mesh->pc colmin): running DVE fp16 tensor-tensor min into a
       per-batch [128, 9056] accumulator (t=0 egresses directly into it).
  The BA accumulator is DMA'd out per batch; the partition-axis (128-way) and
  cross-core mins run on-device in the XLA epilogue; host does the final
  means (a few floats per core).
"""

import sys

sys.path.insert(0, "/opt/trn_rl_repo")

import ml_dtypes
import numpy as np

import concourse.mybir as mybir
from concourse import bacc
from concourse.bass_utils import run_bass_kernel_spmd
from concourse.tile import TileContext

# ---- problem constants (hardcoded; kernel.py must be self-contained) ----
N_BATCH = 2
P = 8192                # point-cloud points per batch
Q = 95 * 95             # 9025 refined mesh points per batch
N_CORES = 8
KDIM = 13               # augmentation slots (hi/lo split product + both norms)

Q_PAD = 9056            # mesh candidates padded (multiple of 32)
RPC = P // N_CORES      # 1024 pc-query rows per core per batch
RT = RPC // 128         # 8 row-tiles per batch
N_RT = N_BATCH * RT     # 16 row-tiles per core
CHUNK = 512             # matmul moving-operand width (ISA max)
PAD_D = 30000.0         # distance injected for pad candidates (< fp16 max)

# Egress groups per row-tile: 8 x 1024 + the 864 tail. The tail always goes
# to DVE; row-tiles in EXTRA_DVE_RT also send their last 1024 group to DVE.
# This splits egress columns so ACT busy == DVE busy (DVE also runs AB + BA).
EGROUP = 1024
EXTRA_DVE_RT = {1, 4, 7, 10, 13}   # 5 of 16 row-tiles
PSUM_W = 1024           # psum tile width (2 banks)
PSUM_BUFS = 4           # 4 x 2 banks = all 8 PSUM banks
E_BUFS = 4
ACC_BUFS = 2
LAG = 2                 # consumers lag this many row-tiles

_F32 = mybir.dt.float32
_F16 = mybir.dt.float16
_BF16 = mybir.dt.bfloat16
_BF16_NP = ml_dtypes.bfloat16


def _build_nc(repeat=1):
    nc = bacc.Bacc("TRN2", target_bir_lowering=False)
    q_d = nc.dram_tensor("queries", [KDIM, N_RT * 128], _BF16, kind="ExternalInput")
    c_d = nc.dram_tensor("cands", [KDIM, N_BATCH * Q_PAD], _BF16, kind="ExternalInput")
    ab_d = nc.dram_tensor("abmins", [128, N_RT], _F32, kind="ExternalOutput")
    ba_d = nc.dram_tensor("bapart", [128, N_BATCH * Q_PAD], _F16, kind="ExternalOutput")

    with TileContext(nc) as tc:
        with (
            tc.tile_pool(name="const", bufs=1) as cpool,
            tc.tile_pool(name="psum", bufs=PSUM_BUFS, space="PSUM") as ppool,
            tc.tile_pool(name="ebuf", bufs=E_BUFS) as epool,
            tc.tile_pool(name="accb", bufs=ACC_BUFS) as apool,
        ):
            qt = cpool.tile([KDIM, N_RT * 128], _BF16)
            ct = cpool.tile([KDIM, N_BATCH * Q_PAD], _BF16)
            # first candidate chunk + queries first, so matmuls start early
            nc.sync.dma_start(out=ct[:, :1024], in_=c_d[:, :1024])
            nc.sync.dma_start(out=qt[:], in_=q_d[:])
            CLOAD = 2264  # Q_PAD / 4
            for b in range(N_BATCH):
                for o in range(0, Q_PAD, CLOAD):
                    s = b * Q_PAD + o
                    w = min(CLOAD, Q_PAD - o)
                    if b == 0 and o == 0:
                        s, w = s + 1024, w - 1024  # already loaded
                    nc.sync.dma_start(
                        out=ct[:, s : s + w], in_=c_d[:, s : s + w]
                    )
            abm = cpool.tile([128, N_RT], _F32)

            pending = []  # deferred consumers: (E, acc, b, t, rt_g)

            HALF = Q_PAD // 2

            def emit_consumers(p):
                E, acc, b, t, rt_g = p
                # AB: one 4x tensor_scalar with min-accum over the row-tile
                nc.vector.tensor_scalar(
                    out=E[:], in0=E[:], scalar1=0.0, scalar2=None,
                    op0=mybir.AluOpType.bypass, op1=mybir.AluOpType.min,
                    accum_out=abm[:, rt_g : rt_g + 1],
                )
                # BA: running fp16 min (t=0 egressed straight into acc).
                # The last row-tile runs in two halves so the output DMA of
                # each half overlaps the other half's min.
                if t < RT - 1:
                    if t > 0:
                        nc.vector.tensor_tensor(
                            acc[:], acc[:], E[:], op=mybir.AluOpType.min
                        )
                else:
                    QTR = Q_PAD // 4
                    for h0 in range(0, Q_PAD, QTR):
                        h1 = min(h0 + QTR, Q_PAD)
                        nc.vector.tensor_tensor(
                            acc[:, h0:h1], acc[:, h0:h1], E[:, h0:h1],
                            op=mybir.AluOpType.min,
                        )
                        nc.sync.dma_start(
                            out=ba_d[:, b * Q_PAD + h0 : b * Q_PAD + h1],
                            in_=acc[:, h0:h1],
                        )

            # per row-tile: (offset, width, engine)
            def groups_for(rt_g):
                gs = []
                off = 0
                n_full = Q_PAD // EGROUP
                for g in range(n_full):
                    eng = "D" if (g == n_full - 1 and rt_g in EXTRA_DVE_RT) else "A"
                    gs.append((off, EGROUP, eng))
                    off += EGROUP
                if Q_PAD % EGROUP:
                    gs.append((off, Q_PAD % EGROUP, "D"))
                return gs

            for _rep in range(repeat):
                for b in range(N_BATCH):
                    acc = apool.tile([128, Q_PAD], _F16, tag="acc")
                    for t in range(RT):
                        rt_g = b * RT + t
                        lhsT = qt[:, rt_g * 128 : (rt_g + 1) * 128]
                        # t=0 egresses straight into the BA accumulator: the
                        # first row-tile IS the initial running min
                        E = acc if t == 0 else epool.tile([128, Q_PAD], _F16, tag="E")
                        for off, w, eng in groups_for(rt_g):
                            ps = ppool.tile([128, PSUM_W], _F32)
                            for k in range(0, w, CHUNK):
                                cw = min(CHUNK, w - k)
                                nc.tensor.matmul(
                                    ps[:, k : k + cw],
                                    lhsT,
                                    ct[:, b * Q_PAD + off + k : b * Q_PAD + off + k + cw],
                                    start=True,
                                    stop=True,
                                )
                            if eng == "A":
                                nc.scalar.copy(out=E[:, off : off + w], in_=ps[:, :w])
                            else:
                                # DVE egress emitted before the lagged
                                # consumers so the PSUM slot recycles ahead
                                # of the big AB/BA ops in the queue
                                nc.vector.tensor_copy(
                                    out=E[:, off : off + w], in_=ps[:, :w]
                                )
                        # consumers lag LAG row-tiles so engine queues never
                        # head-of-line block on a not-yet-egressed E
                        pending.append((E, acc, b, t, rt_g))
                        if len(pending) > LAG:
                            emit_consumers(pending.pop(0))
                while pending:
                    emit_consumers(pending.pop(0))
                nc.sync.dma_start(out=ab_d[:], in_=abm[:])
    nc.compile()
    return nc


_NC_CACHE = None


def _get_nc():
    global _NC_CACHE
    if _NC_CACHE is None:
        _NC_CACHE = _build_nc()
    return _NC_CACHE


class _Runner:
    """Persistent jitted shard_map runner, split into three jits (the
    bass_exec custom call must be alone in its module):

      _zfn   -- makes the donated output buffers on device
      _fn    -- the bass kernel (shard_map over 8 cores)
      _efn   -- epilogue: per-core abmins sum + bapart partition-min, so the
                host downloads ~0.6 MB instead of ~19 MB
    """

    def __init__(self, nc, n_cores=N_CORES):
        import jax
        import jax.numpy as jnp
        from jax.sharding import Mesh, PartitionSpec
        from jax.experimental.shard_map import shard_map
        from concourse import bass2jax

        bass2jax.install_neuronx_cc_hook()
        self._jax = jax
        self.n_cores = n_cores
        part_name = nc.partition_id_tensor.name if nc.partition_id_tensor else None
        in_names, out_names, out_avals, zero_shapes = [], [], [], []
        for alloc in nc.m.functions[0].allocations:
            if not isinstance(alloc, mybir.MemoryLocationSet):
                continue
            name = alloc.memorylocations[0].name
            if alloc.kind == "ExternalInput":
                if name != part_name:
                    in_names.append(name)
            elif alloc.kind == "ExternalOutput":
                out_names.append(name)
                shape = tuple(alloc.tensor_shape)
                dtype = mybir.dt.np(alloc.dtype)
                out_avals.append(jax.core.ShapedArray(shape, dtype))
                zero_shapes.append((shape, dtype))
        self.in_names, self.out_names = in_names, out_names
        n_params = len(in_names)
        all_names = in_names + out_names
        if part_name is not None:
            all_names = all_names + [part_name]
        has_epi = "abmins" in out_names and "bapart" in out_names
        if has_epi:
            self._iab = out_names.index("abmins")
            self._iba = out_names.index("bapart")

        def _body(*args):
            operands = list(args)
            if part_name is not None:
                operands.append(bass2jax.partition_id_tensor())
            return tuple(
                bass2jax._bass_exec_p.bind(
                    *operands,
                    out_avals=tuple(out_avals),
                    in_names=tuple(all_names),
                    out_names=tuple(out_names),
                    lowering_input_output_aliases=(),
                    sim_require_finite=True,
                    sim_require_nnan=True,
                    nc=nc,
                )
            )

        devices = jax.devices()[:n_cores]
        mesh = Mesh(np.asarray(devices), ("core",))
        pcore = PartitionSpec("core")
        self._sharding = jax.sharding.NamedSharding(mesh, pcore)
        n_out = len(out_names)
        self._fn = jax.jit(
            shard_map(
                _body,
                mesh=mesh,
                in_specs=(pcore,) * (n_params + n_out),
                out_specs=(pcore,) * n_out,
                check_rep=False,
            ),
            donate_argnums=tuple(range(n_params, n_params + n_out)),
            keep_unused=True,
        )

        def _zeros():
            return tuple(
                jnp.zeros((n_cores * s[0], *s[1:]), d) for s, d in zero_shapes
            )

        self._zfn = jax.jit(_zeros, out_shardings=(self._sharding,) * n_out)

        if has_epi:
            def _epi(abmins, bapart):
                ab_sum = jnp.sum(abmins.astype(jnp.float32))[None]
                ba = bapart.astype(jnp.float32)
                ba = ba.reshape(128, N_BATCH, Q_PAD).min(axis=0)
                return ab_sum, ba

            self._efn = jax.jit(
                shard_map(
                    _epi, mesh=mesh, in_specs=(pcore, pcore),
                    out_specs=(pcore, pcore), check_rep=False,
                )
            )

    def prepare(self, in_maps):
        """Concatenate per-core inputs along axis 0 (host side)."""
        return [
            np.concatenate([np.asarray(m[name]) for m in in_maps], axis=0)
            for name in self.in_names
        ]

    def run_raw(self, concat_in):
        """bass call only; returns the sharded raw outputs (device)."""
        return self._fn(*concat_in, *self._zfn())

    def run_prepared(self, concat_in):
        outs = self.run_raw(concat_in)
        red = self._efn(outs[self._iab], outs[self._iba])
        self._jax.block_until_ready(red)
        return red

    def __call__(self, in_maps):
        ab_sums, ba = self.run_prepared(self.prepare(in_maps))
        # ab_sums: [n_cores]; ba: [n_cores * N_BATCH, Q_PAD]
        ab_total = float(np.sum(np.asarray(ab_sums, dtype=np.float64)))
        ba = np.asarray(ba, dtype=np.float32).reshape(
            self.n_cores, N_BATCH, Q_PAD
        )
        ba_min = ba.min(axis=0)  # [N_BATCH, Q_PAD]
        return ab_total, ba_min


_RUNNER_CACHE = None


def _get_runner():
    global _RUNNER_CACHE
    if _RUNNER_CACHE is None:
        _RUNNER_CACHE = _Runner(_get_nc())
    return _RUNNER_CACHE


def _upsample_last(x):
    """[..., W] -> [..., 2W-1] midpoint refinement (align_corners=True)."""
    mid = np.float32(0.5) * (x[..., :-1] + x[..., 1:])
    w = x.shape[-1]
    out = np.zeros(x.shape[:-1] + (2 * w - 1,), x.dtype)
    out[..., 0::2] = x
    out[..., 1::2] = mid
    return out


def _split(x):
    """f32 -> (hi, lo) bf16 pair with hi + lo ~= x."""
    h32 = x.astype(_BF16_NP).astype(np.float32)
    lo = (x - h32).astype(_BF16_NP)
    return h32.astype(_BF16_NP), lo


def _fill_queries(dst, pts, n2):
    """dst: [KDIM, n] bf16; pts: [n, 3] f32 queries; n2: [n] query norms."""
    h, l = _split(pts.T)                 # [3, n] each
    dst[0:3] = h
    dst[3:6] = h
    dst[6:9] = l
    dst[9] = _BF16_NP(1.0)
    dst[10] = _BF16_NP(1.0)
    n2h, n2l = _split(n2)
    dst[11] = n2h
    dst[12] = n2l


def _fill_cands(dst, pts, n2):
    """dst: [KDIM, n] bf16; pts: [n, 3] f32 candidates; n2: [n] cand norms."""
    h, l = _split(-2.0 * pts.T)          # exact *(-2) before split
    dst[0:3] = h
    dst[3:6] = l
    dst[6:9] = h
    n2h, n2l = _split(n2)
    dst[9] = n2h
    dst[10] = n2l
    dst[11] = _BF16_NP(1.0)
    dst[12] = _BF16_NP(1.0)


def _prep_inputs(vertices, pc):
    """Host prep: mesh refinement + augmented query/candidate matrices."""
    v = np.asarray(vertices, dtype=np.float32)
    a = np.asarray(pc, dtype=np.float32)                     # [n, P, 3]
    v = _upsample_last(v)                                    # refine W
    v = _upsample_last(v.swapaxes(-1, -2)).swapaxes(-1, -2)  # refine H
    top = v.reshape(N_BATCH, 3, -1).transpose(0, 2, 1)       # [n, Q, 3]

    a2 = np.sum(a * a, axis=-1)        # [n, P]
    b2 = np.sum(top * top, axis=-1)    # [n, Q]

    queries = [
        np.empty((KDIM, N_RT * 128), dtype=_BF16_NP) for _ in range(N_CORES)
    ]
    cands = np.zeros((KDIM, N_BATCH * Q_PAD), dtype=_BF16_NP)
    for b in range(N_BATCH):
        for c in range(N_CORES):
            sl = slice(c * RPC, (c + 1) * RPC)
            _fill_queries(
                queries[c][:, b * RPC : (b + 1) * RPC], a[b, sl], a2[b, sl]
            )
        dst = cands[:, b * Q_PAD : b * Q_PAD + Q]
        _fill_cands(dst, top[b], b2[b])
        # pad candidates: zero coords, huge norm -> never the min
        cands[9, b * Q_PAD + Q : (b + 1) * Q_PAD] = _BF16_NP(PAD_D)
        cands[11, b * Q_PAD + Q : (b + 1) * Q_PAD] = _BF16_NP(1.0)
        cands[12, b * Q_PAD + Q : (b + 1) * Q_PAD] = _BF16_NP(1.0)
    return queries, cands


def kernel(vertices, pc):
    queries, cands = _prep_inputs(vertices, pc)
    in_maps = [{"queries": queries[c], "cands": cands} for c in range(N_CORES)]
    try:
        ab_total, ba_min = _get_runner()(in_maps)
    except Exception:
        ab_total, ba_min = _get_runner()(in_maps)  # retry once
    ab_mean = ab_total / (N_BATCH * P)
    ba_mean = float(np.mean(ba_min[:, :Q].astype(np.float64)))
    return np.float32(ab_mean + ba_mean)


# revision 3
# speedup vs baseline: 1.0857x; 1.0857x over previous
"""Trainium2 Bass kernel v2 for nn_MeshLoss2D (chamfer distance between a
point cloud and a bilinearly-refined mesh).

Contract: kernel(vertices, pc) takes FULL inputs, returns the FULL (scalar)
output. Internally shards across 8 NeuronCores (data-parallel over pc rows).

  chamfer = mean_p min_q d(p,q) + mean_q min_p d(p,q),
  d(p,q) = |a_p|^2 + |b_q|^2 - 2 a_p . b_q

Design (per core, per iteration):
  queries = its 1024 pc rows x 2 batches as 16 row-tiles of 128;
  candidates = the full 9056-padded mesh (9025 refined points + 31 pads).
  Distances via K=13 compensated-bf16 matmuls (hi/lo split).  Each row-tile's
  [128, 9056] f32 PSUM is egressed to fp16 SBUF in five groups:
  4x2048 on ACT + the 864 tail on DVE (this split balances the two engines).
  The fp16 copy E is consumed twice:
    AB (pc->mesh rowmin): one DVE tensor_scalar min-accum (4x_2p fast mode).
    BA (mesh->pc colmin): running DVE fp16 tensor-tensor min into a
       per-batch [128, 9056] accumulator (t=0 egresses directly into it).
  The BA accumulator is DMA'd out per batch; the partition-axis (128-way) and
  cross-core mins run on-device in the XLA epilogue; host does the final
  means (a few floats per core).
"""

import sys

sys.path.insert(0, "/opt/trn_rl_repo")

import ml_dtypes
import numpy as np

import concourse.mybir as mybir
from concourse import bacc
from concourse.bass_utils import run_bass_kernel_spmd
from concourse.tile import TileContext

# ---- problem constants (hardcoded; kernel.py must be self-contained) ----
N_BATCH = 2
P = 8192                # point-cloud points per batch
Q = 95 * 95             # 9025 refined mesh points per batch
N_CORES = 8
KDIM = 13               # augmentation slots (hi/lo split product + both norms)

Q_PAD = 9056            # mesh candidates padded (multiple of 32)
RPC = P // N_CORES      # 1024 pc-query rows per core per batch
RT = RPC // 128         # 8 row-tiles per batch
N_RT = N_BATCH * RT     # 16 row-tiles per core
CHUNK = 512             # matmul moving-operand width (ISA max)
PAD_D = 30000.0         # distance injected for pad candidates (< fp16 max)

# Egress groups per row-tile: 8 x 1024 + the 864 tail. The tail always goes
# to DVE; row-tiles in EXTRA_DVE_RT also send their last 1024 group to DVE.
# This splits egress columns so ACT busy == DVE busy (DVE also runs AB + BA).
EGROUP = 1024
EXTRA_DVE_RT = {1, 4, 7, 10, 13}   # 5 of 16 row-tiles
PSUM_W = 1024           # psum tile width (2 banks)
PSUM_BUFS = 4           # 4 x 2 banks = all 8 PSUM banks
E_BUFS = 4
ACC_BUFS = 2
LAG = 2                 # consumers lag this many row-tiles

_F32 = mybir.dt.float32
_F16 = mybir.dt.float16
_BF16 = mybir.dt.bfloat16
_BF16_NP = ml_dtypes.bfloat16


def _build_nc(repeat=1):
    nc = bacc.Bacc("TRN2", target_bir_lowering=False)
    q_d = nc.dram_tensor("queries", [KDIM, N_RT * 128], _BF16, kind="ExternalInput")
    c_d = nc.dram_tensor("cands", [KDIM, N_BATCH * Q_PAD], _BF16, kind="ExternalInput")
    ab_d = nc.dram_tensor("abmins", [128, N_RT], _F32, kind="ExternalOutput")
    ba_d = nc.dram_tensor("bapart", [128, N_BATCH * Q_PAD], _F16, kind="ExternalOutput")

    with TileContext(nc) as tc:
        with (
            tc.tile_pool(name="const", bufs=1) as cpool,
            tc.tile_pool(name="psum", bufs=PSUM_BUFS, space="PSUM") as ppool,
            tc.tile_pool(name="ebuf", bufs=E_BUFS) as epool,
            tc.tile_pool(name="accb", bufs=ACC_BUFS) as apool,
            tc.tile_pool(name="foldb", bufs=2) as fpool,
        ):
            qt = cpool.tile([KDIM, N_RT * 128], _BF16)
            ct = cpool.tile([KDIM, N_BATCH * Q_PAD], _BF16)
            # first candidate chunk + queries first, so matmuls start early
            nc.sync.dma_start(out=ct[:, :1024], in_=c_d[:, :1024])
            nc.sync.dma_start(out=qt[:], in_=q_d[:])
            CLOAD = 2264  # Q_PAD / 4
            for b in range(N_BATCH):
                for o in range(0, Q_PAD, CLOAD):
                    s = b * Q_PAD + o
                    w = min(CLOAD, Q_PAD - o)
                    if b == 0 and o == 0:
                        s, w = s + 1024, w - 1024  # already loaded
                    nc.sync.dma_start(
                        out=ct[:, s : s + w], in_=c_d[:, s : s + w]
                    )
            abm = cpool.tile([128, N_RT], _F32)

            pending = []  # deferred consumers: (E, acc, b, t, rt_g)

            HALF = Q_PAD // 2

            def emit_consumers(p):
                E, acc, b, t, rt_g = p
                # AB rowmin, folded: one fp16 TT min of the two halves (runs
                # in the DVE 4x fast mode on HW, ~1.7us), then the
                # tensor_scalar+min-accum on half the width.  Measured ~4.7us
                # on HW vs ~7.0us for the single full-width accum op.
                fh = fpool.tile([128, HALF], _F16, tag="fold")
                nc.vector.tensor_tensor(
                    fh[:], E[:, :HALF], E[:, HALF:], op=mybir.AluOpType.min
                )
                nc.vector.tensor_scalar(
                    out=fh[:], in0=fh[:], scalar1=0.0, scalar2=None,
                    op0=mybir.AluOpType.bypass, op1=mybir.AluOpType.min,
                    accum_out=abm[:, rt_g : rt_g + 1],
                )
                # BA: running fp16 min (t=0 egressed straight into acc).
                # The last row-tile runs in two halves so the output DMA of
                # each half overlaps the other half's min.
                if t < RT - 1:
                    if t > 0:
                        nc.vector.tensor_tensor(
                            acc[:], acc[:], E[:], op=mybir.AluOpType.min
                        )
                else:
                    QTR = Q_PAD // 4
                    for h0 in range(0, Q_PAD, QTR):
                        h1 = min(h0 + QTR, Q_PAD)
                        nc.vector.tensor_tensor(
                            acc[:, h0:h1], acc[:, h0:h1], E[:, h0:h1],
                            op=mybir.AluOpType.min,
                        )
                        nc.sync.dma_start(
                            out=ba_d[:, b * Q_PAD + h0 : b * Q_PAD + h1],
                            in_=acc[:, h0:h1],
                        )

            # per row-tile: (offset, width, engine)
            def groups_for(rt_g):
                gs = []
                off = 0
                n_full = Q_PAD // EGROUP
                for g in range(n_full):
                    eng = "D" if (g == n_full - 1 and rt_g in EXTRA_DVE_RT) else "A"
                    gs.append((off, EGROUP, eng))
                    off += EGROUP
                if Q_PAD % EGROUP:
                    gs.append((off, Q_PAD % EGROUP, "D"))
                return gs

            for _rep in range(repeat):
                for b in range(N_BATCH):
                    acc = apool.tile([128, Q_PAD], _F16, tag="acc")
                    for t in range(RT):
                        rt_g = b * RT + t
                        lhsT = qt[:, rt_g * 128 : (rt_g + 1) * 128]
                        # t=0 egresses straight into the BA accumulator: the
                        # first row-tile IS the initial running min
                        E = acc if t == 0 else epool.tile([128, Q_PAD], _F16, tag="E")
                        for off, w, eng in groups_for(rt_g):
                            ps = ppool.tile([128, PSUM_W], _F32)
                            for k in range(0, w, CHUNK):
                                cw = min(CHUNK, w - k)
                                nc.tensor.matmul(
                                    ps[:, k : k + cw],
                                    lhsT,
                                    ct[:, b * Q_PAD + off + k : b * Q_PAD + off + k + cw],
                                    start=True,
                                    stop=True,
                                )
                            if eng == "A":
                                nc.scalar.copy(out=E[:, off : off + w], in_=ps[:, :w])
                            else:
                                # DVE egress emitted before the lagged
                                # consumers so the PSUM slot recycles ahead
                                # of the big AB/BA ops in the queue
                                nc.vector.tensor_copy(
                                    out=E[:, off : off + w], in_=ps[:, :w]
                                )
                        # consumers lag LAG row-tiles so engine queues never
                        # head-of-line block on a not-yet-egressed E
                        pending.append((E, acc, b, t, rt_g))
                        if len(pending) > LAG:
                            emit_consumers(pending.pop(0))
                while pending:
                    emit_consumers(pending.pop(0))
                nc.sync.dma_start(out=ab_d[:], in_=abm[:])
    nc.compile()
    return nc


_NC_CACHE = None


def _get_nc():
    global _NC_CACHE
    if _NC_CACHE is None:
        _NC_CACHE = _build_nc()
    return _NC_CACHE


class _Runner:
    """Persistent jitted shard_map runner, split into three jits (the
    bass_exec custom call must be alone in its module):

      _zfn   -- makes the donated output buffers on device
      _fn    -- the bass kernel (shard_map over 8 cores)
      _efn   -- epilogue: per-core abmins sum + bapart partition-min, so the
                host downloads ~0.6 MB instead of ~19 MB
    """

    def __init__(self, nc, n_cores=N_CORES):
        import jax
        import jax.numpy as jnp
        from jax.sharding import Mesh, PartitionSpec
        from jax.experimental.shard_map import shard_map
        from concourse import bass2jax

        bass2jax.install_neuronx_cc_hook()
        self._jax = jax
        self.n_cores = n_cores
        part_name = nc.partition_id_tensor.name if nc.partition_id_tensor else None
        in_names, out_names, out_avals, zero_shapes = [], [], [], []
        for alloc in nc.m.functions[0].allocations:
            if not isinstance(alloc, mybir.MemoryLocationSet):
                continue
            name = alloc.memorylocations[0].name
            if alloc.kind == "ExternalInput":
                if name != part_name:
                    in_names.append(name)
            elif alloc.kind == "ExternalOutput":
                out_names.append(name)
                shape = tuple(alloc.tensor_shape)
                dtype = mybir.dt.np(alloc.dtype)
                out_avals.append(jax.core.ShapedArray(shape, dtype))
                zero_shapes.append((shape, dtype))
        self.in_names, self.out_names = in_names, out_names
        n_params = len(in_names)
        all_names = in_names + out_names
        if part_name is not None:
            all_names = all_names + [part_name]
        has_epi = "abmins" in out_names and "bapart" in out_names
        if has_epi:
            self._iab = out_names.index("abmins")
            self._iba = out_names.index("bapart")

        def _body(*args):
            operands = list(args)
            if part_name is not None:
                operands.append(bass2jax.partition_id_tensor())
            return tuple(
                bass2jax._bass_exec_p.bind(
                    *operands,
                    out_avals=tuple(out_avals),
                    in_names=tuple(all_names),
                    out_names=tuple(out_names),
                    lowering_input_output_aliases=(),
                    sim_require_finite=True,
                    sim_require_nnan=True,
                    nc=nc,
                )
            )

        devices = jax.devices()[:n_cores]
        mesh = Mesh(np.asarray(devices), ("core",))
        pcore = PartitionSpec("core")
        self._sharding = jax.sharding.NamedSharding(mesh, pcore)
        n_out = len(out_names)
        self._fn = jax.jit(
            shard_map(
                _body,
                mesh=mesh,
                in_specs=(pcore,) * (n_params + n_out),
                out_specs=(pcore,) * n_out,
                check_rep=False,
            ),
            donate_argnums=tuple(range(n_params, n_params + n_out)),
            keep_unused=True,
        )

        def _zeros():
            return tuple(
                jnp.zeros((n_cores * s[0], *s[1:]), d) for s, d in zero_shapes
            )

        self._zfn = jax.jit(_zeros, out_shardings=(self._sharding,) * n_out)

        if has_epi:
            def _epi(abmins, bapart):
                ab_sum = jnp.sum(abmins.astype(jnp.float32))[None]
                ba = bapart.astype(jnp.float32)
                ba = ba.reshape(128, N_BATCH, Q_PAD).min(axis=0)
                return ab_sum, ba

            self._efn = jax.jit(
                shard_map(
                    _epi, mesh=mesh, in_specs=(pcore, pcore),
                    out_specs=(pcore, pcore), check_rep=False,
                )
            )

    def prepare(self, in_maps):
        """Concatenate per-core inputs along axis 0 (host side)."""
        return [
            np.concatenate([np.asarray(m[name]) for m in in_maps], axis=0)
            for name in self.in_names
        ]

    def run_raw(self, concat_in):
        """bass call only; returns the sharded raw outputs (device)."""
        return self._fn(*concat_in, *self._zfn())

    def run_prepared(self, concat_in):
        outs = self.run_raw(concat_in)
        red = self._efn(outs[self._iab], outs[self._iba])
        self._jax.block_until_ready(red)
        return red

    def __call__(self, in_maps):
        ab_sums, ba = self.run_prepared(self.prepare(in_maps))
        # ab_sums: [n_cores]; ba: [n_cores * N_BATCH, Q_PAD]
        ab_total = float(np.sum(np.asarray(ab_sums, dtype=np.float64)))
        ba = np.asarray(ba, dtype=np.float32).reshape(
            self.n_cores, N_BATCH, Q_PAD
        )
        ba_min = ba.min(axis=0)  # [N_BATCH, Q_PAD]
        return ab_total, ba_min


_RUNNER_CACHE = None


def _get_runner():
    global _RUNNER_CACHE
    if _RUNNER_CACHE is None:
        _RUNNER_CACHE = _Runner(_get_nc())
    return _RUNNER_CACHE


def _upsample_last(x):
    """[..., W] -> [..., 2W-1] midpoint refinement (align_corners=True)."""
    mid = np.float32(0.5) * (x[..., :-1] + x[..., 1:])
    w = x.shape[-1]
    out = np.zeros(x.shape[:-1] + (2 * w - 1,), x.dtype)
    out[..., 0::2] = x
    out[..., 1::2] = mid
    return out


def _split(x):
    """f32 -> (hi, lo) bf16 pair with hi + lo ~= x."""
    h32 = x.astype(_BF16_NP).astype(np.float32)
    lo = (x - h32).astype(_BF16_NP)
    return h32.astype(_BF16_NP), lo


def _fill_queries(dst, pts, n2):
    """dst: [KDIM, n] bf16; pts: [n, 3] f32 queries; n2: [n] query norms."""
    h, l = _split(pts.T)                 # [3, n] each
    dst[0:3] = h
    dst[3:6] = h
    dst[6:9] = l
    dst[9] = _BF16_NP(1.0)
    dst[10] = _BF16_NP(1.0)
    n2h, n2l = _split(n2)
    dst[11] = n2h
    dst[12] = n2l


def _fill_cands(dst, pts, n2):
    """dst: [KDIM, n] bf16; pts: [n, 3] f32 candidates; n2: [n] cand norms."""
    h, l = _split(-2.0 * pts.T)          # exact *(-2) before split
    dst[0:3] = h
    dst[3:6] = l
    dst[6:9] = h
    n2h, n2l = _split(n2)
    dst[9] = n2h
    dst[10] = n2l
    dst[11] = _BF16_NP(1.0)
    dst[12] = _BF16_NP(1.0)


def _prep_inputs(vertices, pc):
    """Host prep: mesh refinement + augmented query/candidate matrices."""
    v = np.asarray(vertices, dtype=np.float32)
    a = np.asarray(pc, dtype=np.float32)                     # [n, P, 3]
    v = _upsample_last(v)                                    # refine W
    v = _upsample_last(v.swapaxes(-1, -2)).swapaxes(-1, -2)  # refine H
    top = v.reshape(N_BATCH, 3, -1).transpose(0, 2, 1)       # [n, Q, 3]

    a2 = np.sum(a * a, axis=-1)        # [n, P]
    b2 = np.sum(top * top, axis=-1)    # [n, Q]

    queries = [
        np.empty((KDIM, N_RT * 128), dtype=_BF16_NP) for _ in range(N_CORES)
    ]
    cands = np.zeros((KDIM, N_BATCH * Q_PAD), dtype=_BF16_NP)
    for b in range(N_BATCH):
        for c in range(N_CORES):
            sl = slice(c * RPC, (c + 1) * RPC)
            _fill_queries(
                queries[c][:, b * RPC : (b + 1) * RPC], a[b, sl], a2[b, sl]
            )
        dst = cands[:, b * Q_PAD : b * Q_PAD + Q]
        _fill_cands(dst, top[b], b2[b])
        # pad candidates: zero coords, huge norm -> never the min
        cands[9, b * Q_PAD + Q : (b + 1) * Q_PAD] = _BF16_NP(PAD_D)
        cands[11, b * Q_PAD + Q : (b + 1) * Q_PAD] = _BF16_NP(1.0)
        cands[12, b * Q_PAD + Q : (b + 1) * Q_PAD] = _BF16_NP(1.0)
    return queries, cands


def kernel(vertices, pc):
    queries, cands = _prep_inputs(vertices, pc)
    in_maps = [{"queries": queries[c], "cands": cands} for c in range(N_CORES)]
    try:
        ab_total, ba_min = _get_runner()(in_maps)
    except Exception:
        ab_total, ba_min = _get_runner()(in_maps)  # retry once
    ab_mean = ab_total / (N_BATCH * P)
    ba_mean = float(np.mean(ba_min[:, :Q].astype(np.float64)))
    return np.float32(ab_mean + ba_mean)


# revision 4
# speedup vs baseline: 1.1347x; 1.0452x over previous
"""Trainium2 Bass kernel v2 for nn_MeshLoss2D (chamfer distance between a
point cloud and a bilinearly-refined mesh).

Contract: kernel(vertices, pc) takes FULL inputs, returns the FULL (scalar)
output. Internally shards across 8 NeuronCores (data-parallel over pc rows).

  chamfer = mean_p min_q d(p,q) + mean_q min_p d(p,q),
  d(p,q) = |a_p|^2 + |b_q|^2 - 2 a_p . b_q

Design (per core, per iteration):
  queries = its 1024 pc rows x 2 batches as 16 row-tiles of 128;
  candidates = the full 9056-padded mesh (9025 refined points + 31 pads).
  Distances via K=13 compensated-bf16 matmuls (hi/lo split).  Each row-tile's
  [128, 9056] f32 PSUM is egressed to fp16 SBUF in five groups:
  4x2048 on ACT + the 864 tail on DVE (this split balances the two engines).
  The fp16 copy E is consumed twice:
    AB (pc->mesh rowmin): one DVE tensor_scalar min-accum (4x_2p fast mode).
    BA (mesh->pc colmin): running DVE fp16 tensor-tensor min into a
       per-batch [128, 9056] accumulator (t=0 egresses directly into it).
  The BA accumulator is DMA'd out per batch; the partition-axis (128-way) and
  cross-core mins run on-device in the XLA epilogue; host does the final
  means (a few floats per core).
"""

import sys

sys.path.insert(0, "/opt/trn_rl_repo")

import ml_dtypes
import numpy as np

import concourse.mybir as mybir
from concourse import bacc
from concourse.bass_utils import run_bass_kernel_spmd
from concourse.tile import TileContext

# ---- problem constants (hardcoded; kernel.py must be self-contained) ----
N_BATCH = 2
P = 8192                # point-cloud points per batch
Q = 95 * 95             # 9025 refined mesh points per batch
N_CORES = 8
KDIM = 13               # augmentation slots (hi/lo split product + both norms)

Q_PAD = 9056            # mesh candidates padded (multiple of 32)
RPC = P // N_CORES      # 1024 pc-query rows per core per batch
RT = RPC // 128         # 8 row-tiles per batch
N_RT = N_BATCH * RT     # 16 row-tiles per core
CHUNK = 512             # matmul moving-operand width (ISA max)
PAD_D = 30000.0         # distance injected for pad candidates (< fp16 max)

# Egress groups per row-tile: 8 x 1024 + the 864 tail. The tail always goes
# to DVE; row-tiles in EXTRA_DVE_RT also send their last 1024 group to DVE.
# This splits egress columns so ACT busy == DVE busy (DVE also runs AB + BA).
EGROUP = 1024
EXTRA_DVE_RT = {4, 12}             # 2 of 16 row-tiles
PSUM_W = 1024           # psum tile width (2 banks)
PSUM_BUFS = 4           # 4 x 2 banks = all 8 PSUM banks
E_BUFS = 5
ACC_BUFS = 2
LAG = 3                 # consumers lag this many row-tiles

_F32 = mybir.dt.float32
_F16 = mybir.dt.float16
_BF16 = mybir.dt.bfloat16
_BF16_NP = ml_dtypes.bfloat16


def _build_nc(repeat=1):
    nc = bacc.Bacc("TRN2", target_bir_lowering=False)
    q_d = nc.dram_tensor("queries", [KDIM, N_RT * 128], _BF16, kind="ExternalInput")
    c_d = nc.dram_tensor("cands", [KDIM, N_BATCH * Q_PAD], _BF16, kind="ExternalInput")
    ab_d = nc.dram_tensor("abmins", [128, N_RT], _F32, kind="ExternalOutput")
    ba_d = nc.dram_tensor("bapart", [128, N_BATCH * Q_PAD], _F16, kind="ExternalOutput")

    with TileContext(nc) as tc:
        with (
            tc.tile_pool(name="const", bufs=1) as cpool,
            tc.tile_pool(name="psum", bufs=PSUM_BUFS, space="PSUM") as ppool,
            tc.tile_pool(name="ebuf", bufs=E_BUFS) as epool,
            tc.tile_pool(name="accb", bufs=ACC_BUFS) as apool,
            tc.tile_pool(name="foldb", bufs=2) as fpool,
        ):
            qt = cpool.tile([KDIM, N_RT * 128], _BF16)
            ct = cpool.tile([KDIM, N_BATCH * Q_PAD], _BF16)
            # first candidate chunk + queries first, so matmuls start early
            nc.sync.dma_start(out=ct[:, :1024], in_=c_d[:, :1024])
            nc.sync.dma_start(out=qt[:], in_=q_d[:])
            CLOAD = 2264  # Q_PAD / 4
            for b in range(N_BATCH):
                for o in range(0, Q_PAD, CLOAD):
                    s = b * Q_PAD + o
                    w = min(CLOAD, Q_PAD - o)
                    if b == 0 and o == 0:
                        s, w = s + 1024, w - 1024  # already loaded
                    nc.sync.dma_start(
                        out=ct[:, s : s + w], in_=c_d[:, s : s + w]
                    )
            abm = cpool.tile([128, N_RT], _F32)

            pending = []  # deferred consumers: (E, acc, b, t, rt_g)

            HALF = Q_PAD // 2

            def emit_consumers(p):
                E, acc, b, t, rt_g = p
                # AB rowmin, folded: one fp16 TT min of the two halves (runs
                # in the DVE 4x fast mode on HW, ~1.7us), then the
                # tensor_scalar+min-accum on half the width.  Measured ~4.7us
                # on HW vs ~7.0us for the single full-width accum op.
                fh = fpool.tile([128, HALF], _F16, tag="fold")
                nc.vector.tensor_tensor(
                    fh[:], E[:, :HALF], E[:, HALF:], op=mybir.AluOpType.min
                )
                nc.vector.tensor_scalar(
                    out=fh[:], in0=fh[:], scalar1=0.0, scalar2=None,
                    op0=mybir.AluOpType.bypass, op1=mybir.AluOpType.min,
                    accum_out=abm[:, rt_g : rt_g + 1],
                )
                # BA: running fp16 min (t=0 egressed straight into acc).
                # The last row-tile runs in two halves so the output DMA of
                # each half overlaps the other half's min.
                if t < RT - 1:
                    if t > 0:
                        nc.vector.tensor_tensor(
                            acc[:], acc[:], E[:], op=mybir.AluOpType.min
                        )
                else:
                    QTR = Q_PAD // 4
                    for h0 in range(0, Q_PAD, QTR):
                        h1 = min(h0 + QTR, Q_PAD)
                        nc.vector.tensor_tensor(
                            acc[:, h0:h1], acc[:, h0:h1], E[:, h0:h1],
                            op=mybir.AluOpType.min,
                        )
                        nc.sync.dma_start(
                            out=ba_d[:, b * Q_PAD + h0 : b * Q_PAD + h1],
                            in_=acc[:, h0:h1],
                        )

            # per row-tile: (offset, width, engine)
            def groups_for(rt_g):
                gs = []
                off = 0
                n_full = Q_PAD // EGROUP
                for g in range(n_full):
                    eng = "D" if (g == n_full - 1 and rt_g in EXTRA_DVE_RT) else "A"
                    gs.append((off, EGROUP, eng))
                    off += EGROUP
                if Q_PAD % EGROUP:
                    gs.append((off, Q_PAD % EGROUP, "D"))
                return gs

            for _rep in range(repeat):
                for b in range(N_BATCH):
                    acc = apool.tile([128, Q_PAD], _F16, tag="acc")
                    for t in range(RT):
                        rt_g = b * RT + t
                        lhsT = qt[:, rt_g * 128 : (rt_g + 1) * 128]
                        # t=0 egresses straight into the BA accumulator: the
                        # first row-tile IS the initial running min
                        E = acc if t == 0 else epool.tile([128, Q_PAD], _F16, tag="E")
                        for off, w, eng in groups_for(rt_g):
                            ps = ppool.tile([128, PSUM_W], _F32)
                            for k in range(0, w, CHUNK):
                                cw = min(CHUNK, w - k)
                                nc.tensor.matmul(
                                    ps[:, k : k + cw],
                                    lhsT,
                                    ct[:, b * Q_PAD + off + k : b * Q_PAD + off + k + cw],
                                    start=True,
                                    stop=True,
                                )
                            if eng == "A":
                                nc.scalar.copy(out=E[:, off : off + w], in_=ps[:, :w])
                            else:
                                # DVE egress emitted before the lagged
                                # consumers so the PSUM slot recycles ahead
                                # of the big AB/BA ops in the queue
                                nc.vector.tensor_copy(
                                    out=E[:, off : off + w], in_=ps[:, :w]
                                )
                        # consumers lag LAG row-tiles so engine queues never
                        # head-of-line block on a not-yet-egressed E
                        pending.append((E, acc, b, t, rt_g))
                        if len(pending) > LAG:
                            emit_consumers(pending.pop(0))
                while pending:
                    emit_consumers(pending.pop(0))
                nc.sync.dma_start(out=ab_d[:], in_=abm[:])
    nc.compile()
    return nc


_NC_CACHE = None


def _get_nc():
    global _NC_CACHE
    if _NC_CACHE is None:
        _NC_CACHE = _build_nc()
    return _NC_CACHE


class _Runner:
    """Persistent jitted shard_map runner, split into three jits (the
    bass_exec custom call must be alone in its module):

      _zfn   -- makes the donated output buffers on device
      _fn    -- the bass kernel (shard_map over 8 cores)
      _efn   -- epilogue: per-core abmins sum + bapart partition-min, so the
                host downloads ~0.6 MB instead of ~19 MB
    """

    def __init__(self, nc, n_cores=N_CORES):
        import jax
        import jax.numpy as jnp
        from jax.sharding import Mesh, PartitionSpec
        from jax.experimental.shard_map import shard_map
        from concourse import bass2jax

        bass2jax.install_neuronx_cc_hook()
        self._jax = jax
        self.n_cores = n_cores
        part_name = nc.partition_id_tensor.name if nc.partition_id_tensor else None
        in_names, out_names, out_avals, zero_shapes = [], [], [], []
        for alloc in nc.m.functions[0].allocations:
            if not isinstance(alloc, mybir.MemoryLocationSet):
                continue
            name = alloc.memorylocations[0].name
            if alloc.kind == "ExternalInput":
                if name != part_name:
                    in_names.append(name)
            elif alloc.kind == "ExternalOutput":
                out_names.append(name)
                shape = tuple(alloc.tensor_shape)
                dtype = mybir.dt.np(alloc.dtype)
                out_avals.append(jax.core.ShapedArray(shape, dtype))
                zero_shapes.append((shape, dtype))
        self.in_names, self.out_names = in_names, out_names
        n_params = len(in_names)
        all_names = in_names + out_names
        if part_name is not None:
            all_names = all_names + [part_name]
        has_epi = "abmins" in out_names and "bapart" in out_names
        if has_epi:
            self._iab = out_names.index("abmins")
            self._iba = out_names.index("bapart")

        def _body(*args):
            operands = list(args)
            if part_name is not None:
                operands.append(bass2jax.partition_id_tensor())
            return tuple(
                bass2jax._bass_exec_p.bind(
                    *operands,
                    out_avals=tuple(out_avals),
                    in_names=tuple(all_names),
                    out_names=tuple(out_names),
                    lowering_input_output_aliases=(),
                    sim_require_finite=True,
                    sim_require_nnan=True,
                    nc=nc,
                )
            )

        devices = jax.devices()[:n_cores]
        mesh = Mesh(np.asarray(devices), ("core",))
        pcore = PartitionSpec("core")
        self._sharding = jax.sharding.NamedSharding(mesh, pcore)
        n_out = len(out_names)
        self._fn = jax.jit(
            shard_map(
                _body,
                mesh=mesh,
                in_specs=(pcore,) * (n_params + n_out),
                out_specs=(pcore,) * n_out,
                check_rep=False,
            ),
            donate_argnums=tuple(range(n_params, n_params + n_out)),
            keep_unused=True,
        )

        def _zeros():
            return tuple(
                jnp.zeros((n_cores * s[0], *s[1:]), d) for s, d in zero_shapes
            )

        self._zfn = jax.jit(_zeros, out_shardings=(self._sharding,) * n_out)

        if has_epi:
            def _epi(abmins, bapart):
                ab_sum = jnp.sum(abmins.astype(jnp.float32))[None]
                ba = bapart.astype(jnp.float32)
                ba = ba.reshape(128, N_BATCH, Q_PAD).min(axis=0)
                return ab_sum, ba

            self._efn = jax.jit(
                shard_map(
                    _epi, mesh=mesh, in_specs=(pcore, pcore),
                    out_specs=(pcore, pcore), check_rep=False,
                )
            )

    def prepare(self, in_maps):
        """Concatenate per-core inputs along axis 0 (host side)."""
        return [
            np.concatenate([np.asarray(m[name]) for m in in_maps], axis=0)
            for name in self.in_names
        ]

    def run_raw(self, concat_in):
        """bass call only; returns the sharded raw outputs (device)."""
        return self._fn(*concat_in, *self._zfn())

    def run_prepared(self, concat_in):
        outs = self.run_raw(concat_in)
        red = self._efn(outs[self._iab], outs[self._iba])
        self._jax.block_until_ready(red)
        return red

    def __call__(self, in_maps):
        ab_sums, ba = self.run_prepared(self.prepare(in_maps))
        # ab_sums: [n_cores]; ba: [n_cores * N_BATCH, Q_PAD]
        ab_total = float(np.sum(np.asarray(ab_sums, dtype=np.float64)))
        ba = np.asarray(ba, dtype=np.float32).reshape(
            self.n_cores, N_BATCH, Q_PAD
        )
        ba_min = ba.min(axis=0)  # [N_BATCH, Q_PAD]
        return ab_total, ba_min


_RUNNER_CACHE = None


def _get_runner():
    global _RUNNER_CACHE
    if _RUNNER_CACHE is None:
        _RUNNER_CACHE = _Runner(_get_nc())
    return _RUNNER_CACHE


def _upsample_last(x):
    """[..., W] -> [..., 2W-1] midpoint refinement (align_corners=True)."""
    mid = np.float32(0.5) * (x[..., :-1] + x[..., 1:])
    w = x.shape[-1]
    out = np.zeros(x.shape[:-1] + (2 * w - 1,), x.dtype)
    out[..., 0::2] = x
    out[..., 1::2] = mid
    return out


def _split(x):
    """f32 -> (hi, lo) bf16 pair with hi + lo ~= x."""
    h32 = x.astype(_BF16_NP).astype(np.float32)
    lo = (x - h32).astype(_BF16_NP)
    return h32.astype(_BF16_NP), lo


def _fill_queries(dst, pts, n2):
    """dst: [KDIM, n] bf16; pts: [n, 3] f32 queries; n2: [n] query norms."""
    h, l = _split(pts.T)                 # [3, n] each
    dst[0:3] = h
    dst[3:6] = h
    dst[6:9] = l
    dst[9] = _BF16_NP(1.0)
    dst[10] = _BF16_NP(1.0)
    n2h, n2l = _split(n2)
    dst[11] = n2h
    dst[12] = n2l


def _fill_cands(dst, pts, n2):
    """dst: [KDIM, n] bf16; pts: [n, 3] f32 candidates; n2: [n] cand norms."""
    h, l = _split(-2.0 * pts.T)          # exact *(-2) before split
    dst[0:3] = h
    dst[3:6] = l
    dst[6:9] = h
    n2h, n2l = _split(n2)
    dst[9] = n2h
    dst[10] = n2l
    dst[11] = _BF16_NP(1.0)
    dst[12] = _BF16_NP(1.0)


def _prep_inputs(vertices, pc):
    """Host prep: mesh refinement + augmented query/candidate matrices."""
    v = np.asarray(vertices, dtype=np.float32)
    a = np.asarray(pc, dtype=np.float32)                     # [n, P, 3]
    v = _upsample_last(v)                                    # refine W
    v = _upsample_last(v.swapaxes(-1, -2)).swapaxes(-1, -2)  # refine H
    top = v.reshape(N_BATCH, 3, -1).transpose(0, 2, 1)       # [n, Q, 3]

    a2 = np.sum(a * a, axis=-1)        # [n, P]
    b2 = np.sum(top * top, axis=-1)    # [n, Q]

    queries = [
        np.empty((KDIM, N_RT * 128), dtype=_BF16_NP) for _ in range(N_CORES)
    ]
    cands = np.zeros((KDIM, N_BATCH * Q_PAD), dtype=_BF16_NP)
    for b in range(N_BATCH):
        for c in range(N_CORES):
            sl = slice(c * RPC, (c + 1) * RPC)
            _fill_queries(
                queries[c][:, b * RPC : (b + 1) * RPC], a[b, sl], a2[b, sl]
            )
        dst = cands[:, b * Q_PAD : b * Q_PAD + Q]
        _fill_cands(dst, top[b], b2[b])
        # pad candidates: zero coords, huge norm -> never the min
        cands[9, b * Q_PAD + Q : (b + 1) * Q_PAD] = _BF16_NP(PAD_D)
        cands[11, b * Q_PAD + Q : (b + 1) * Q_PAD] = _BF16_NP(1.0)
        cands[12, b * Q_PAD + Q : (b + 1) * Q_PAD] = _BF16_NP(1.0)
    return queries, cands


def kernel(vertices, pc):
    queries, cands = _prep_inputs(vertices, pc)
    in_maps = [{"queries": queries[c], "cands": cands} for c in range(N_CORES)]
    try:
        ab_total, ba_min = _get_runner()(in_maps)
    except Exception:
        ab_total, ba_min = _get_runner()(in_maps)  # retry once
    ab_mean = ab_total / (N_BATCH * P)
    ba_mean = float(np.mean(ba_min[:, :Q].astype(np.float64)))
    return np.float32(ab_mean + ba_mean)
